# revision 54
# baseline (speedup 1.0000x reference)
"""Conv1d-QKV + full attention kernel for TRN2, 8 NeuronCores — fp8 DoubleRow.

Problem (hardcoded): B=4, S=4096, DIN=DQ=DK=256.
  q = conv1d(query, q_w, q_b); k = conv1d(key, ...); v = conv1d(value, ...)
  out = scale * softmax(q @ k^T / sqrt(256)) @ v

Sharding: 8 cores = (batch b = core//2) x (query half h = core%2); each core
convs k/v over the full 4096 rows of its batch and runs attention for its
2048 query rows.

Numerics (validated vs reference in fp-sim + hw):
  - all matmuls fp8e4 DoubleRow (two 128-chunks folded per matmul, 0.5
    cycles/output-col in the cost model = 4x over f32r).
  - q/k convs: 2-term fp8 residual (w8x8 + w8xr8); v conv: 3-term
    (+ wr8x8) since v errors reach the output unaveraged.
  - scores: q,k quantized to fp8 at 2^6; exp on Act with the combined
    descale folded into the activation scale.
  - e path: centered quantization ec8 = fp8(32*(exp(s)-1)); the matching
    correction c*colsum(v8+vr8) is folded into the final DVE pass (not a
    PE rank-1): t1 = po + corrb, out = t1*rec + scale*bias.
  - v path: 2-term fp8 residual (v8 + vr8) cast straight from the conv
    PSUM (v8 = ps*scl11 on Act/DVE; vr8 = (ps*scl11) - v8 as one fused
    scalar_tensor_tensor).  The conv bias is NOT added on-device: since
    softmax rows sum to 1, att@(v+b) = att@v + b, so scale*b rides the
    final stt as a broadcast add.
  - softmax denominator rides as an extra v_aug column (value 64 = v scale).

Engine split: exp on Act only (the only engine with activation tables);
ec center-casts spread over DVE/Pool/Act by CFG["ec_pat"] (GPSIMD cannot
read PSUM, so every PSUM-reading cast is on DVE); q/k conv casts and
v8/vr8 on DVE.  Score pairs are paced through the conv and out sections
(CFG scheds) so the Act exp stream and the PE never starve each other;
ec casts are emission-deferred (drain_ec) to place their engine-queue
slots in low-pressure regions.

PSUM pools are phase-scoped (A/B: 4-deep conv ring + 2-deep scores;
C: conv_v + scores + out; D: 3-deep scores + out) — open order matters:
a pool inherits the banks of the previously-closed pool at the same
address, so its first tiles wait on that pool's last consumers.
"""

import numpy as np

B, S, DIN, D = 4, 4096, 256, 256
NCORES = 8
SQ = S // 2
NKT = S // 128        # 32 key tiles
NKP = NKT // 2        # 16 key-tile pairs
CT = 512              # conv column tile
XT_ST = 528           # conv x tile stride (mult of 16 for fp8 DoubleRow)
VW = 272              # v_aug row stride (mult of 16); data 0:256, den col 256
OC = 257              # out matmul columns (256 data + denominator)
QB = 512              # query block
NBLK = SQ // QB       # 4

SX = 32.0             # x scale 2^5
SW = 4096.0           # w scale 2^12
SQK = 64.0            # q/k fp8 scale 2^6
SV = 64.0             # v fp8 scale 2^6
SE = 32.0             # e fp8 scale 2^5
EXP_SCALE = 1.0 / (16.0 * 4096.0)   # 1/16 softmax scale / (2^6*2^6 qk scale)

CFG = {
    # engine for each score-pair center-cast, keyed (blk, kp):
    # A=Act, D=DVE, P=Pool.  (GPSIMD cannot read PSUM, so only these
    # SBUF->SBUF casts may use P.)  Late block-3 casts avoid P: the Pool
    # queue runs ~10us behind and the tail out halves would wait on it.
    "ec_pat": {
        0: "PPPPPPPPPPPPPPPP",
        1: "PDPPDPPDPPDPPPDP",
        2: "DPPDPDPPDPPDPDPP",
        3: "DDDDDDDD" + "DDDDDDAD",
    },
    # conv psum->fp8 casts in emission order (PSUM read: D only)
    "qk_pat": "D" * 12,
    # v8 cast engine per merged k-tile pair (16); PSUM read: A or D only
    "v8_pat": "DDDDDDDDDDDDDDDD",
    "ps_sc_bufs": 2,
    "ps_o_bufs": 2,
    "ps_cv_bufs": 2,
    "xin_bufs": 8,
    "et_bufs": 30,
    "outp_bufs": 6,
    "t1_bufs": 4,
    # pairs emitted per k-conv tile (8 entries, block 0 then block 1)
    "kc_dist": [2, 2, 2, 2, 2, 2, 2, 2],
    # pairs emitted per v-conv tile (8 entries)
    "vc_dist": [2, 2, 2, 2, 2, 2, 2, 2],
}


def _split_drain_waits(nc):
    """Walrus accepts one sem-wait per CTRL (Drain) instruction; split any
    multi-wait Drain into a chain of single-wait no-ops."""
    import concourse.mybir as mybir

    def walk(blocks):
        for b in blocks:
            insts = b.instructions
            i = 0
            while i < len(insts):
                inst = insts[i]
                si = getattr(inst, "sync_info", None)
                w = list(si.on_wait) if si is not None and si.on_wait else []
                if len(w) > 1:
                    pre = [
                        mybir.InstNoOp(
                            name=f"{inst.name}-ws{j}",
                            engine=inst.engine,
                            ins=[],
                            outs=[],
                            sync_info=mybir.SyncInfo(on_wait=[wj], on_update=[]),
                        )
                        for j, wj in enumerate(w[:-1])
                    ]
                    si.on_wait = w[-1:]
                    for k, nd in enumerate(pre):
                        insts.insert(i + k, nd)
                    i += len(pre)
                i += 1
            walk(getattr(b, "blocks", []) or [])

    for f in nc.m.functions:
        walk(f.blocks)


def _build_bass():
    import concourse.bass as bass
    import concourse.mybir as mybir
    import concourse.tile as tile

    f32 = mybir.dt.float32
    bf16 = mybir.dt.bfloat16
    fp8 = mybir.dt.float8e4
    DR = mybir.MatmulPerfMode.DoubleRow
    ADD = mybir.AluOpType.add
    SUB = mybir.AluOpType.subtract
    MULT = mybir.AluOpType.mult
    Exp = mybir.ActivationFunctionType.Exp
    Copy = mybir.ActivationFunctionType.Copy

    nc = bass.Bass(trn_type="TRN2")

    # ---- DRAM I/O (per-core shard shapes) ----
    # x tiles: dim2 slots 0:2 = x8 (a-chunks), 2:4 = xr8
    xq = nc.dram_tensor("xq", [SQ // CT, 128, 4, XT_ST], fp8, kind="ExternalInput")
    xk = nc.dram_tensor("xk", [S // CT, 128, 4, XT_ST], fp8, kind="ExternalInput")
    xv = nc.dram_tensor("xv", [S // CT, 128, 4, XT_ST], fp8, kind="ExternalInput")
    # T-layout weights (w8 only: the 2-term q/k conv never reads wr8):
    # [p, a, t, o, m]
    wq = nc.dram_tensor("wq", [128, 2, 3, 2, 128], fp8, kind="ExternalInput")
    wk = nc.dram_tensor("wk", [128, 2, 3, 2, 128], fp8, kind="ExternalInput")
    # natural-layout v weights: [p, w/wr, a, t, co]
    wv = nc.dram_tensor("wv", [128, 2, 2, 3, D], fp8, kind="ExternalInput")
    bqk = nc.dram_tensor("bqk", [128, 4], f32, kind="ExternalInput")
    aux = nc.dram_tensor("aux", [128, 1 + D], f32, kind="ExternalInput")
    ones8 = nc.dram_tensor("ones8", [128, 2, 128], fp8, kind="ExternalInput")
    vtop8 = nc.dram_tensor("vtop8", [128, NKT, VW - D], fp8, kind="ExternalInput")
    vtop0 = nc.dram_tensor("vtop0", [128, NKT, VW - D], fp8, kind="ExternalInput")
    out = nc.dram_tensor("out", [SQ // 128, 128, D], f32, kind="ExternalOutput")

    ec_pat = CFG["ec_pat"]
    qk_pat = CFG["qk_pat"]
    v8_pat = CFG["v8_pat"]

    with tile.TileContext(nc) as tc:
        with (
            tc.tile_pool(name="persist", bufs=1) as persist,
            tc.tile_pool(name="xin", bufs=CFG["xin_bufs"]) as xin,
            tc.tile_pool(name="et", bufs=CFG["et_bufs"]) as etp,
            tc.tile_pool(name="outp", bufs=CFG["outp_bufs"]) as outp,
            tc.tile_pool(name="t1p", bufs=CFG["t1_bufs"]) as t1p,
            tc.tile_pool(name="tiny", bufs=4) as tiny,
        ):
            wq_s = persist.tile([128, 2, 3, 2, 128], fp8, tag="wq_s")
            wk_s = persist.tile([128, 2, 3, 2, 128], fp8, tag="wk_s")
            wv_s = persist.tile([128, 2, 2, 3, D], fp8, tag="wv_s")
            bqk_s = persist.tile([128, 4], f32, tag="bqk_s")
            aux_s = persist.tile([128, 1 + D], f32, tag="aux_s")
            ones8_s = persist.tile([128, 2, 128], fp8, tag="ones8_s")
            qT8_s = persist.tile([128, 2, SQ], fp8, tag="qT8_s")
            kT8_s = persist.tile([128, 2, S], fp8, tag="kT8_s")
            v8_s = persist.tile([128, NKT, VW], fp8, tag="v8_s")
            vr8_s = persist.tile([128, NKT, VW], fp8, tag="vr8_s")
            corrb_s = persist.tile([128, OC], f32, tag="corrb_s")
            ec8_s = persist.tile([128, NBLK, NKT, QB], fp8, tag="ec8_s")

            # fp8 residual terms (w-residual, x-residual); qk convs run
            # 2-term (w8x8 + w8xr8), the v conv keeps all 3 terms since v
            # errors reach the output unaveraged.
            TERMS_QK = [(0, 0), (0, 1)]
            TERMS_V = [(0, 0), (0, 1), (1, 0)]

            qk_cast_i = [0]

            def conv_T(x_dram, w_s, b_s, out_s, j, cv_pool, dma_eng=None,
                       split_first=False):
                """T-layout conv tile j; per-o psum, cast (with bias) on
                DVE.  The 4-deep conv pool hides the cast latency."""
                xt = xin.tile([128, 4, XT_ST], fp8, tag="xt")
                if split_first:
                    (dma_eng or nc.sync).dma_start(xt[:, 0:2], x_dram[j, :, 0:2])
                    (dma_eng or nc.sync).dma_start(xt[:, 2:4], x_dram[j, :, 2:4])
                else:
                    (dma_eng or nc.sync).dma_start(xt[:], x_dram[j])
                for o in range(2):
                    ps = cv_pool.tile([128, CT], f32, tag="cv")
                    i = 0
                    for rw, rx in TERMS_QK:
                        for t in range(3):
                            nc.tensor.matmul(
                                ps[:],
                                w_s[:, :, t, o, :],
                                xt[:, 2 * rx : 2 * rx + 2, t : t + CT],
                                start=(i == 0),
                                stop=(i == 5),
                                perf_mode=DR,
                            )
                            i += 1
                    nc.vector.tensor_scalar(
                        out=out_s[:, o, j * CT : (j + 1) * CT], in0=ps[:],
                        scalar1=2.0**-11, scalar2=b_s[:, o : o + 1],
                        op0=MULT, op1=ADD)

            xv_pre = {}

            def xv_fetch(j):
                xt = xin.tile([128, 4, XT_ST], fp8, tag="xt")
                nc.sync.dma_start(xt[:], xv[j])
                xv_pre[j] = xt

            def conv_v(j, cv_pool, halves=(0, 1)):
                """natural-layout v conv for x tile j; two k-tiles share one
                psum tile so the casts process 2x elements per instruction:
                v8 = fp8(ps*scl11), vr8 = fp8(ps*scl11 - v8) fused stt."""
                if j not in xv_pre:
                    xt = xin.tile([128, 4, XT_ST], fp8, tag="xt")
                    nc.sync.dma_start(xt[:], xv[j])
                    xv_pre[j] = xt
                xt = xv_pre[j]
                if 1 in halves:
                    xv_pre.pop(j)
                for half in halves:
                    kt0 = j * (CT // 128) + 2 * half
                    ps = cv_pool.tile([128, CT], f32, tag="ps_cv")
                    for r in range(2):
                        first = True
                        for rw, rx in TERMS_V:
                            for t in range(3):
                                s0 = t + (2 * half + r) * 128
                                nc.tensor.matmul(
                                    ps[:, r * D : (r + 1) * D],
                                    xt[:, 2 * rx : 2 * rx + 2, s0 : s0 + 128],
                                    wv_s[:, rw, :, t, :],
                                    start=first,
                                    stop=(rw, rx) == TERMS_V[-1] and t == 2,
                                    perf_mode=DR,
                                )
                                first = False
                    dst8 = v8_s[:, kt0 : kt0 + 2, 0:D]
                    eng = v8_pat[kt0 // 2]
                    if eng == "A":
                        nc.scalar.activation(out=dst8, in_=ps[:], func=Copy,
                                             scale=aux_s[:, 0:1])
                    else:
                        nc.vector.tensor_scalar(out=dst8, in0=ps[:],
                                                scalar1=aux_s[:, 0:1],
                                                scalar2=None, op0=MULT)
                    nc.vector.scalar_tensor_tensor(
                        out=vr8_s[:, kt0 : kt0 + 2, 0:D], in0=ps[:],
                        scalar=aux_s[:, 0:1], in1=dst8,
                        op0=MULT, op1=SUB)

            pending_ec = []
            cur_sc = [None]  # phase-scoped scores/conv PSUM pool

            def scores_pair(blk, kp):
                """scores^T for key tiles 2kp,2kp+1 vs query block blk + exp;
                the fp8 center-cast is deferred (drain_ec)."""
                q0 = blk * QB
                ps = cur_sc[0].tile([128, 2, QB], f32, tag="big")
                for i in range(2):
                    kt = 2 * kp + i
                    nc.tensor.matmul(
                        ps[:, i, :],
                        kT8_s[:, :, kt * 128 : (kt + 1) * 128],
                        qT8_s[:, :, q0 : q0 + QB],
                        start=True,
                        stop=True,
                        perf_mode=DR,
                    )
                et = etp.tile([128, 2, QB], bf16, tag="et")
                nc.scalar.activation(out=et[:], in_=ps[:], func=Exp,
                                     scale=EXP_SCALE)
                pending_ec.append((blk, kp, et))

            def drain_ec(n=None):
                """emit n (default all) pending ec center-casts."""
                m = len(pending_ec) if n is None else min(n, len(pending_ec))
                for _ in range(m):
                    blk, kp, et = pending_ec.pop(0)
                    dst = ec8_s[:, blk, 2 * kp : 2 * kp + 2, :]
                    eng = ec_pat[blk][kp]
                    if eng == "A":
                        nc.scalar.activation(out=dst, in_=et[:], func=Copy,
                                             scale=SE, bias=-SE)
                    elif eng == "P":
                        nc.gpsimd.tensor_scalar(out=dst, in0=et[:], scalar1=SE,
                                                scalar2=-SE, op0=MULT, op1=ADD)
                    else:
                        nc.vector.tensor_scalar(out=dst, in0=et[:], scalar1=SE,
                                                scalar2=-SE, op0=MULT, op1=ADD)

            po_open = {}

            def out_half(blk, qs, part, split_dma=False):
                """Half of po's accumulation: part 0 = kp 0..7, part 1 =
                kp 8..15 + DVE epilogue (corr add, reciprocal, scale+bias,
                DMA out)."""
                if part == 0:
                    po = ps_o.tile([128, OC], f32, tag="ps_o")
                    po_open[(blk, qs)] = po
                    kps = range(0, NKP // 2)
                    first = True
                else:
                    po = po_open.pop((blk, qs))
                    kps = range(NKP // 2, NKP)
                    first = False
                for vs in (v8_s, vr8_s):
                    for kp in kps:
                        nc.tensor.matmul(
                            po[:],
                            ec8_s[:, blk, 2 * kp : 2 * kp + 2,
                                  qs * 128 : (qs + 1) * 128],
                            vs[:, 2 * kp : 2 * kp + 2, 0:OC],
                            start=first,
                            stop=(part == 1 and vs is vr8_s and kp == NKP - 1),
                            perf_mode=DR,
                        )
                        first = False
                if part == 0:
                    return
                t1 = t1p.tile([128, OC], f32, tag="t1")
                nc.vector.tensor_tensor(out=t1[:], in0=po[:], in1=corrb_s[:],
                                        op=ADD)
                rec = tiny.tile([128, 1], f32, tag="rec")
                nc.vector.reciprocal(rec[:], t1[:, 256:257])
                ot = outp.tile([128, D], f32, tag="ot")
                row = blk * (QB // 128) + qs
                if not split_dma:
                    nc.vector.scalar_tensor_tensor(
                        out=ot[:], in0=t1[:, 0:D], scalar=rec[:],
                        in1=aux_s[:, 1 : 1 + D], op0=MULT, op1=ADD)
                    nc.sync.dma_start(out[row], ot[:])
                else:
                    # final tiles: halve the exposed epilogue+DMA tail by
                    # overlapping the two column halves on two queues
                    nc.vector.scalar_tensor_tensor(
                        out=ot[:, 0:128], in0=t1[:, 0:128], scalar=rec[:],
                        in1=aux_s[:, 1:129], op0=MULT, op1=ADD)
                    nc.scalar.dma_start(out[row, :, 0:128], ot[:, 0:128])
                    nc.vector.scalar_tensor_tensor(
                        out=ot[:, 128:D], in0=t1[:, 128:D], scalar=rec[:],
                        in1=aux_s[:, 129 : 1 + D], op0=MULT, op1=ADD)
                    nc.sync.dma_start(out[row, :, 128:D], ot[:, 128:D])

            # ---- emission order (software pipeline) ----
            # Prime the Act Exp table at t=0 so the first real exp does not
            # pay the 1283ns table load on the critical path.
            warm = tiny.tile([128, 1], f32, tag="warm")
            nc.vector.memset(warm[:], 0.0)
            warm2 = tiny.tile([128, 1], bf16, tag="warm2")
            nc.scalar.activation(out=warm2[:], in_=warm[:], func=Exp)

            # critical-path DMAs first: wq split so the first matmul only
            # waits on the w8 half; weights on the SP HWDGE queue, x tiles
            # on the Act queue so the transfers overlap.
            nc.sync.dma_start(wq_s[:], wq[:])
            nc.sync.dma_start(bqk_s[:], bqk[:])
            nc.sync.dma_start(wk_s[:], wk[:])

            # Pair schedule: a pair (blk, kp) may be emitted once k-conv
            # tile kp//2 is done (and q-conv tile blk).  Pairs are spread so
            # Act's exp demand tracks each section's wall: 20 in A/B (blocks
            # 0 + early 1), 20 in C (blocks 1 + early 2), 24 in D.
            # iteration j of A/B: [leader (already-enabled pair, emitted
            # between q-conv and k-conv), new pairs (0,2j), (0,2j+1)]
            sched_ab = [
                [(0, 0), (0, 1)],
                [(1, 0), (0, 2), (0, 3)],
                [(1, 1), (0, 4), (0, 5)],
                [(1, 2), (0, 6), (0, 7)],
                [(1, 3), (0, 8), (0, 9)],
                [(1, 4), (0, 10), (0, 11)],
                [(1, 5), (0, 12), (0, 13)],
                [(1, 6), (0, 14), (0, 15)],
            ]
            sched_c = [
                [(1, 7), (1, 8)],
                [(1, 9), (1, 10), (1, 11)],
                [(1, 12), (1, 13)],
                [(1, 14), (1, 15), (2, 0)],
                [(2, 1), (2, 2)],
                [(2, 3), (2, 4), (2, 5)],
                [(2, 6), (2, 7)],
                [(2, 8), (2, 9), (2, 10)],
            ]
            sched_d = ([(2, kp) for kp in range(11, NKP)]
                       + [(3, kp) for kp in range(NKP)])

            cvT_cm = tc.tile_pool(name="ps_ab_cv", bufs=4, space="PSUM")
            ab_sc = tc.tile_pool(name="ps_ab_sc", bufs=2, space="PSUM")
            cvT = cvT_cm.__enter__()
            cur_sc[0] = ab_sc.__enter__()
            if True:
                conv_T(xq, wq_s, bqk_s[:, 0:2], qT8_s, 0, cvT,
                       dma_eng=nc.scalar, split_first=True)
                conv_T(xk, wk_s, bqk_s[:, 2:4], kT8_s, 0, cvT,
                       dma_eng=nc.scalar)
                for p in sched_ab[0]:
                    scores_pair(*p)
                drain_ec(1)
                for j in range(1, 4):
                    conv_T(xq, wq_s, bqk_s[:, 0:2], qT8_s, j, cvT,
                           dma_eng=nc.scalar)
                    scores_pair(*sched_ab[j][0])  # leader: old k tiles only
                    conv_T(xk, wk_s, bqk_s[:, 2:4], kT8_s, j, cvT)
                    for p in sched_ab[j][1:]:
                        scores_pair(*p)
                    drain_ec(len(sched_ab[j]))
                nc.sync.dma_start(wv_s[:], wv[:])
                nc.sync.dma_start(aux_s[:], aux[:])
                xv_fetch(0)
                xv_fetch(1)
                nc.sync.dma_start(ones8_s[:], ones8[:])
                nc.sync.dma_start(v8_s[:, :, D:VW], vtop8[:])
                nc.sync.dma_start(vr8_s[:, :, D:VW], vtop0[:])
                for j in range(4, 8):
                    scores_pair(*sched_ab[j][0])
                    conv_T(xk, wk_s, bqk_s[:, 2:4], kT8_s, j, cvT)
                    for p in sched_ab[j][1:]:
                        scores_pair(*p)
                    drain_ec(len(sched_ab[j]))
                xv_fetch(2)
                xv_fetch(3)
            ab_sc.__exit__(None, None, None)
            cvT_cm.__exit__(None, None, None)

            # C: v conv + pairs (blocks 1-2); their ec casts partially
            # deferred into D.  Once v8/vr8 tiles 0..15 exist (after conv_v
            # j3), block-0 part0 out-halves start filling PE's idle slots.
            ps_o_cm = tc.tile_pool(name="ps_o", bufs=CFG["ps_o_bufs"],
                                   space="PSUM")
            cvV_cm = tc.tile_pool(name="ps_cvV", bufs=CFG["ps_cv_bufs"],
                                  space="PSUM")
            scC_cm = tc.tile_pool(name="ps_c_sc", bufs=2, space="PSUM")
            ps_o = ps_o_cm.__enter__()
            cvV = cvV_cm.__enter__()
            scC = scC_cm.__enter__()
            if True:
                cur_sc[0] = scC
                for j in range(8):
                    sc = sched_c[j]
                    scores_pair(*sc[0])
                    conv_v(j, cvV)
                    for p in sc[1:]:
                        scores_pair(*p)
                    drain_ec(2)
                    if j == 5:
                        out_half(0, 0, 0)
                    if j == 7:
                        out_half(0, 1, 0)

                # correction: colsum(v8+vr8) replicated over rows, scaled by SE
                pc = cvV.tile([128, CT], f32, tag="ps_cv")
                first = True
                for vs in (v8_s, vr8_s):
                    for kp in range(NKP):
                        nc.tensor.matmul(pc[:, 0:OC], ones8_s[:],
                                         vs[:, 2 * kp : 2 * kp + 2, 0:OC],
                                         start=first,
                                         stop=(vs is vr8_s and kp == NKP - 1),
                                         perf_mode=DR)
                        first = False
                nc.vector.tensor_scalar(out=corrb_s[:], in0=pc[:, 0:OC],
                                        scalar1=SE, scalar2=None, op0=MULT)

            # D: out halves with the remaining pairs interleaved singly (a
            # lone pair between halves never outruns Act by more than the
            # ps_sc depth).  The staggered part0/part1 chain keeps exactly
            # two po tiles open (= ps_o bufs); block-3 part1 halves (which
            # need pair (3,15)) come last.
            scC_cm.__exit__(None, None, None)
            cvV_cm.__exit__(None, None, None)
            ps_d = tc.tile_pool(name="ps_d", bufs=3, space="PSUM")
            cur_sc[0] = ps_d.__enter__()
            di = 0

            def dpair():
                nonlocal di
                if di < len(sched_d):
                    scores_pair(*sched_d[di])
                    di += 1
                    drain_ec(2)

            chain = [
                ((0, 0, 1), (0, 2, 0)), ((0, 1, 1), (0, 3, 0)),
                ((0, 2, 1), (1, 0, 0)), ((0, 3, 1), (1, 1, 0)),
                ((1, 0, 1), (1, 2, 0)), ((1, 1, 1), (1, 3, 0)),
                ((1, 2, 1), (2, 0, 0)), ((1, 3, 1), (2, 1, 0)),
                ((2, 0, 1), (2, 2, 0)), ((2, 1, 1), (2, 3, 0)),
                ((2, 2, 1), (3, 0, 0)), ((2, 3, 1), (3, 1, 0)),
            ]
            for h1, h2 in chain:
                dpair()
                out_half(*h1)
                dpair()
                out_half(*h2)
            for _ in range(len(sched_d) - di):
                dpair()
            drain_ec()
            ps_d.__exit__(None, None, None)
            out_half(3, 0, 1)
            out_half(3, 2, 0)
            out_half(3, 1, 1)
            out_half(3, 3, 0)
            out_half(3, 2, 1)
            out_half(3, 3, 1)
            ps_o_cm.__exit__(None, None, None)

    _split_drain_waits(nc)
    return nc


_NC_CACHE = None


def _get_nc():
    global _NC_CACHE
    if _NC_CACHE is None:
        _NC_CACHE = _build_bass()
    return _NC_CACHE


def _fp8(a):
    import ml_dtypes
    return np.asarray(np.clip(a, -240.0, 240.0), ml_dtypes.float8_e4m3)


def _xtiles(x_pad):
    """[128, 2, n+2] f32 -> fp8 2-term tiles [nj, 128, 4, 528]."""
    n = x_pad.shape[2] - 2
    nj = n // CT
    x8 = _fp8(x_pad * SX)
    xr8 = _fp8(x_pad * SX - x8.astype(np.float32))
    tiles = np.zeros((nj, 128, 4, XT_ST), x8.dtype)
    for j in range(nj):
        sl = slice(j * CT, j * CT + CT + 2)
        tiles[j, :, 0:2, 0 : CT + 2] = x8[:, :, sl]
        tiles[j, :, 2:4, 0 : CT + 2] = xr8[:, :, sl]
    return tiles


def _xT_padded(x_b):
    """[S, C] -> transposed + halo-padded [128, 2, S+2] f32."""
    xt = np.zeros((DIN, x_b.shape[0] + 2), np.float32)
    xt[:, 1:-1] = x_b.T
    return np.ascontiguousarray(
        xt.reshape(2, 128, x_b.shape[0] + 2).transpose(1, 0, 2)
    )


def _w2(w_scaled):
    """scaled f32 weights -> (w8, wr8) fp8 pair."""
    w8 = _fp8(w_scaled)
    wr8 = _fp8(w_scaled - w8.astype(np.float32))
    return w8, wr8


def _prep_shared(q_w, q_b, k_w, k_b, v_w, v_b, scale):
    import ml_dtypes
    FP8 = ml_dtypes.float8_e4m3

    def w_T(w):  # [co, ci, 3] -> [p, a, t, o, m] f32
        arr = np.ascontiguousarray(w.transpose(1, 2, 0))  # [ci, t, co]
        arr = arr.reshape(2, 128, 3, 2, 128)  # [a, p, t, o, m]
        return np.ascontiguousarray(arr.transpose(1, 0, 2, 3, 4)).astype(np.float32)

    def w_v(w):  # [co, ci, 3] -> [p, a, t, co] f32
        arr = np.ascontiguousarray(w.transpose(1, 2, 0))
        arr = arr.reshape(2, 128, 3, D)
        return np.ascontiguousarray(arr.transpose(1, 0, 2, 3)).astype(np.float32)

    def pack_T(w):
        return _fp8(w_T(w) * SW)

    wv8, wvr8 = _w2(w_v(v_w) * SW)
    vtop8 = np.zeros((128, NKT, VW - D), FP8)
    vtop8[:, :, 0] = FP8(SV)
    return {
        "wq": pack_T(q_w),
        "wk": pack_T(k_w),
        "wv": np.ascontiguousarray(np.stack([wv8, wvr8], axis=1)),
        "bqk": np.concatenate([
            np.ascontiguousarray(q_b.reshape(2, 128).T).astype(np.float32),
            np.ascontiguousarray(k_b.reshape(2, 128).T).astype(np.float32),
        ], axis=1) * SQK,
        "aux": np.concatenate([
            np.full((128, 1), float(scale) * 2.0**-11, np.float32),
            np.tile(v_b.astype(np.float32)[None, :] * float(scale), (128, 1)),
        ], axis=1),
        "ones8": np.ones((128, 2, 128), FP8),
        "vtop8": vtop8,
        "vtop0": np.zeros((128, NKT, VW - D), FP8),
    }


def kernel(query, key, value, q_w, q_b, k_w, k_b, v_w, v_b, scale):
    from concourse.bass_utils import run_bass_kernel_spmd

    query = np.asarray(query, np.float32)
    key = np.asarray(key, np.float32)
    value = np.asarray(value, np.float32)

    shared = _prep_shared(
        np.asarray(q_w), np.asarray(q_b), np.asarray(k_w), np.asarray(k_b),
        np.asarray(v_w), np.asarray(v_b), np.asarray(scale),
    )

    in_maps = []
    for c in range(NCORES):
        b, h = c // 2, c % 2
        xq_full = _xT_padded(query[b])  # [128, 2, S+2]
        xq_c = np.ascontiguousarray(xq_full[:, :, h * SQ : h * SQ + SQ + 2])
        m = dict(shared)
        m["xq"] = _xtiles(xq_c)
        m["xk"] = _xtiles(_xT_padded(key[b]))
        m["xv"] = _xtiles(_xT_padded(value[b]))
        in_maps.append(m)

    nc = _get_nc()
    res = run_bass_kernel_spmd(nc, in_maps, core_ids=list(range(NCORES)))

    out_full = np.empty((B, S, D), np.float32)
    for c in range(NCORES):
        b, h = c // 2, c % 2
        out_full[b, h * SQ : (h + 1) * SQ, :] = res.results[c]["out"].reshape(SQ, D)
    return out_full


# revision 55
# speedup vs baseline: 1.0043x; 1.0043x over previous
"""Conv1d-QKV + full attention kernel for TRN2, 8 NeuronCores — fp8 DoubleRow.

Problem (hardcoded): B=4, S=4096, DIN=DQ=DK=256.
  q = conv1d(query, q_w, q_b); k = conv1d(key, ...); v = conv1d(value, ...)
  out = scale * softmax(q @ k^T / sqrt(256)) @ v

Sharding: 8 cores = (batch b = core//2) x (query half h = core%2); each core
convs k/v over the full 4096 rows of its batch and runs attention for its
2048 query rows.

Numerics (validated vs reference in fp-sim + hw):
  - all matmuls fp8e4 DoubleRow (two 128-chunks folded per matmul, 0.5
    cycles/output-col in the cost model = 4x over f32r).
  - q/k convs: 2-term fp8 residual (w8x8 + w8xr8); v conv: 3-term
    (+ wr8x8) since v errors reach the output unaveraged.
  - scores: q,k quantized to fp8 at 2^6; exp on Act with the combined
    descale folded into the activation scale.
  - e path: centered quantization ec8 = fp8(32*(exp(s)-1)); the matching
    correction c*colsum(v8+vr8) is folded into the final DVE pass (not a
    PE rank-1): t1 = po + corrb, out = t1*rec + scale*bias.
  - v path: 2-term fp8 residual (v8 + vr8) cast straight from the conv
    PSUM (v8 = ps*scl11 on Act/DVE; vr8 = (ps*scl11) - v8 as one fused
    scalar_tensor_tensor).  The conv bias is NOT added on-device: since
    softmax rows sum to 1, att@(v+b) = att@v + b, so scale*b rides the
    final stt as a broadcast add.
  - softmax denominator rides as an extra v_aug column (value 64 = v scale).

Engine split: exp on Act only (the only engine with activation tables);
ec center-casts spread over DVE/Pool/Act by CFG["ec_pat"] (GPSIMD cannot
read PSUM, so every PSUM-reading cast is on DVE); q/k conv casts and
v8/vr8 on DVE.  Score pairs are paced through the conv and out sections
(CFG scheds) so the Act exp stream and the PE never starve each other;
ec casts are emission-deferred (drain_ec) to place their engine-queue
slots in low-pressure regions.

PSUM pools are phase-scoped (A/B: 4-deep conv ring + 2-deep scores;
C: conv_v + scores + out; D: 3-deep scores + out) — open order matters:
a pool inherits the banks of the previously-closed pool at the same
address, so its first tiles wait on that pool's last consumers.
"""

import numpy as np

B, S, DIN, D = 4, 4096, 256, 256
NCORES = 8
SQ = S // 2
NKT = S // 128        # 32 key tiles
NKP = NKT // 2        # 16 key-tile pairs
CT = 512              # conv column tile
XT_ST = 528           # conv x tile stride (mult of 16 for fp8 DoubleRow)
VW = 272              # v_aug row stride (mult of 16); data 0:256, den col 256
OC = 257              # out matmul columns (256 data + denominator)
QB = 512              # query block
NBLK = SQ // QB       # 4

SX = 32.0             # x scale 2^5
SW = 4096.0           # w scale 2^12
SQK = 64.0            # q/k fp8 scale 2^6
SV = 64.0             # v fp8 scale 2^6
SE = 32.0             # e fp8 scale 2^5
EXP_SCALE = 1.0 / (16.0 * 4096.0)   # 1/16 softmax scale / (2^6*2^6 qk scale)

CFG = {
    # engine for each score-pair center-cast, keyed (blk, kp):
    # A=Act, D=DVE, P=Pool.  (GPSIMD cannot read PSUM, so only these
    # SBUF->SBUF casts may use P.)  Late block-3 casts avoid P: the Pool
    # queue runs ~10us behind and the tail out halves would wait on it.
    "ec_pat": {
        0: "PPPPPPPPPPPPPPPP",
        1: "PDPPDPPDPPDPPPDP",
        2: "DPPDPDPPDPPDPDPP",
        3: "DPDPDPDP" + "DDDDDDAD",
    },
    # conv psum->fp8 casts in emission order (PSUM read: D only)
    "qk_pat": "D" * 12,
    # v8 cast engine per merged k-tile pair (16); PSUM read: A or D only
    "v8_pat": "DDDDDDDDDDDDDDDD",
    "ps_sc_bufs": 2,
    "ps_o_bufs": 2,
    "ps_cv_bufs": 2,
    "xin_bufs": 8,
    "et_bufs": 30,
    "outp_bufs": 6,
    "t1_bufs": 4,
    # pairs emitted per k-conv tile (8 entries, block 0 then block 1)
    "kc_dist": [2, 2, 2, 2, 2, 2, 2, 2],
    # pairs emitted per v-conv tile (8 entries)
    "vc_dist": [2, 2, 2, 2, 2, 2, 2, 2],
}


def _split_drain_waits(nc):
    """Walrus accepts one sem-wait per CTRL (Drain) instruction; split any
    multi-wait Drain into a chain of single-wait no-ops."""
    import concourse.mybir as mybir

    def walk(blocks):
        for b in blocks:
            insts = b.instructions
            i = 0
            while i < len(insts):
                inst = insts[i]
                si = getattr(inst, "sync_info", None)
                w = list(si.on_wait) if si is not None and si.on_wait else []
                if len(w) > 1:
                    pre = [
                        mybir.InstNoOp(
                            name=f"{inst.name}-ws{j}",
                            engine=inst.engine,
                            ins=[],
                            outs=[],
                            sync_info=mybir.SyncInfo(on_wait=[wj], on_update=[]),
                        )
                        for j, wj in enumerate(w[:-1])
                    ]
                    si.on_wait = w[-1:]
                    for k, nd in enumerate(pre):
                        insts.insert(i + k, nd)
                    i += len(pre)
                i += 1
            walk(getattr(b, "blocks", []) or [])

    for f in nc.m.functions:
        walk(f.blocks)


def _build_bass():
    import concourse.bass as bass
    import concourse.mybir as mybir
    import concourse.tile as tile

    f32 = mybir.dt.float32
    bf16 = mybir.dt.bfloat16
    fp8 = mybir.dt.float8e4
    DR = mybir.MatmulPerfMode.DoubleRow
    ADD = mybir.AluOpType.add
    SUB = mybir.AluOpType.subtract
    MULT = mybir.AluOpType.mult
    Exp = mybir.ActivationFunctionType.Exp
    Copy = mybir.ActivationFunctionType.Copy

    nc = bass.Bass(trn_type="TRN2")

    # ---- DRAM I/O (per-core shard shapes) ----
    # x tiles: dim2 slots 0:2 = x8 (a-chunks), 2:4 = xr8
    xq = nc.dram_tensor("xq", [SQ // CT, 128, 4, XT_ST], fp8, kind="ExternalInput")
    xk = nc.dram_tensor("xk", [S // CT, 128, 4, XT_ST], fp8, kind="ExternalInput")
    xv = nc.dram_tensor("xv", [S // CT, 128, 4, XT_ST], fp8, kind="ExternalInput")
    # T-layout weights (w8 only: the 2-term q/k conv never reads wr8):
    # [p, a, t, o, m]
    wq = nc.dram_tensor("wq", [128, 2, 3, 2, 128], fp8, kind="ExternalInput")
    wk = nc.dram_tensor("wk", [128, 2, 3, 2, 128], fp8, kind="ExternalInput")
    # natural-layout v weights: [p, w/wr, a, t, co]
    wv = nc.dram_tensor("wv", [128, 2, 2, 3, D], fp8, kind="ExternalInput")
    bqk = nc.dram_tensor("bqk", [128, 4], f32, kind="ExternalInput")
    aux = nc.dram_tensor("aux", [128, 1 + D], f32, kind="ExternalInput")
    ones8 = nc.dram_tensor("ones8", [128, 2, 128], fp8, kind="ExternalInput")
    vtop8 = nc.dram_tensor("vtop8", [128, NKT, VW - D], fp8, kind="ExternalInput")
    vtop0 = nc.dram_tensor("vtop0", [128, NKT, VW - D], fp8, kind="ExternalInput")
    out = nc.dram_tensor("out", [SQ // 128, 128, D], f32, kind="ExternalOutput")

    ec_pat = CFG["ec_pat"]
    qk_pat = CFG["qk_pat"]
    v8_pat = CFG["v8_pat"]

    with tile.TileContext(nc) as tc:
        with (
            tc.tile_pool(name="persist", bufs=1) as persist,
            tc.tile_pool(name="xin", bufs=CFG["xin_bufs"]) as xin,
            tc.tile_pool(name="et", bufs=CFG["et_bufs"]) as etp,
            tc.tile_pool(name="outp", bufs=CFG["outp_bufs"]) as outp,
            tc.tile_pool(name="t1p", bufs=CFG["t1_bufs"]) as t1p,
            tc.tile_pool(name="tiny", bufs=4) as tiny,
        ):
            wq_s = persist.tile([128, 2, 3, 2, 128], fp8, tag="wq_s")
            wk_s = persist.tile([128, 2, 3, 2, 128], fp8, tag="wk_s")
            wv_s = persist.tile([128, 2, 2, 3, D], fp8, tag="wv_s")
            bqk_s = persist.tile([128, 4], f32, tag="bqk_s")
            aux_s = persist.tile([128, 1 + D], f32, tag="aux_s")
            ones8_s = persist.tile([128, 2, 128], fp8, tag="ones8_s")
            qT8_s = persist.tile([128, 2, SQ], fp8, tag="qT8_s")
            kT8_s = persist.tile([128, 2, S], fp8, tag="kT8_s")
            v8_s = persist.tile([128, NKT, VW], fp8, tag="v8_s")
            vr8_s = persist.tile([128, NKT, VW], fp8, tag="vr8_s")
            corrb_s = persist.tile([128, OC], f32, tag="corrb_s")
            ec8_s = persist.tile([128, NBLK, NKT, QB], fp8, tag="ec8_s")

            # fp8 residual terms (w-residual, x-residual); qk convs run
            # 2-term (w8x8 + w8xr8), the v conv keeps all 3 terms since v
            # errors reach the output unaveraged.
            TERMS_QK = [(0, 0), (0, 1)]
            TERMS_V = [(0, 0), (0, 1), (1, 0)]

            qk_cast_i = [0]

            def conv_T(x_dram, w_s, b_s, out_s, j, cv_pool, dma_eng=None,
                       split_first=False):
                """T-layout conv tile j; per-o psum, cast (with bias) on
                DVE.  The 4-deep conv pool hides the cast latency."""
                xt = xin.tile([128, 4, XT_ST], fp8, tag="xt")
                if split_first:
                    (dma_eng or nc.sync).dma_start(xt[:, 0:2], x_dram[j, :, 0:2])
                    (dma_eng or nc.sync).dma_start(xt[:, 2:4], x_dram[j, :, 2:4])
                else:
                    (dma_eng or nc.sync).dma_start(xt[:], x_dram[j])
                for o in range(2):
                    ps = cv_pool.tile([128, CT], f32, tag="cv")
                    i = 0
                    for rw, rx in TERMS_QK:
                        for t in range(3):
                            nc.tensor.matmul(
                                ps[:],
                                w_s[:, :, t, o, :],
                                xt[:, 2 * rx : 2 * rx + 2, t : t + CT],
                                start=(i == 0),
                                stop=(i == 5),
                                perf_mode=DR,
                            )
                            i += 1
                    nc.vector.tensor_scalar(
                        out=out_s[:, o, j * CT : (j + 1) * CT], in0=ps[:],
                        scalar1=2.0**-11, scalar2=b_s[:, o : o + 1],
                        op0=MULT, op1=ADD)

            xv_pre = {}

            def xv_fetch(j):
                xt = xin.tile([128, 4, XT_ST], fp8, tag="xt")
                nc.sync.dma_start(xt[:], xv[j])
                xv_pre[j] = xt

            def conv_v(j, cv_pool, halves=(0, 1)):
                """natural-layout v conv for x tile j; two k-tiles share one
                psum tile so the casts process 2x elements per instruction:
                v8 = fp8(ps*scl11), vr8 = fp8(ps*scl11 - v8) fused stt."""
                if j not in xv_pre:
                    xt = xin.tile([128, 4, XT_ST], fp8, tag="xt")
                    nc.sync.dma_start(xt[:], xv[j])
                    xv_pre[j] = xt
                xt = xv_pre[j]
                if 1 in halves:
                    xv_pre.pop(j)
                for half in halves:
                    kt0 = j * (CT // 128) + 2 * half
                    ps = cv_pool.tile([128, CT], f32, tag="ps_cv")
                    for r in range(2):
                        first = True
                        for rw, rx in TERMS_V:
                            for t in range(3):
                                s0 = t + (2 * half + r) * 128
                                nc.tensor.matmul(
                                    ps[:, r * D : (r + 1) * D],
                                    xt[:, 2 * rx : 2 * rx + 2, s0 : s0 + 128],
                                    wv_s[:, rw, :, t, :],
                                    start=first,
                                    stop=(rw, rx) == TERMS_V[-1] and t == 2,
                                    perf_mode=DR,
                                )
                                first = False
                    dst8 = v8_s[:, kt0 : kt0 + 2, 0:D]
                    eng = v8_pat[kt0 // 2]
                    if eng == "A":
                        nc.scalar.activation(out=dst8, in_=ps[:], func=Copy,
                                             scale=aux_s[:, 0:1])
                    else:
                        nc.vector.tensor_scalar(out=dst8, in0=ps[:],
                                                scalar1=aux_s[:, 0:1],
                                                scalar2=None, op0=MULT)
                    nc.vector.scalar_tensor_tensor(
                        out=vr8_s[:, kt0 : kt0 + 2, 0:D], in0=ps[:],
                        scalar=aux_s[:, 0:1], in1=dst8,
                        op0=MULT, op1=SUB)

            pending_ec = []
            cur_sc = [None]  # phase-scoped scores/conv PSUM pool

            def scores_pair(blk, kp):
                """scores^T for key tiles 2kp,2kp+1 vs query block blk + exp;
                the fp8 center-cast is deferred (drain_ec)."""
                q0 = blk * QB
                ps = cur_sc[0].tile([128, 2, QB], f32, tag="big")
                for i in range(2):
                    kt = 2 * kp + i
                    nc.tensor.matmul(
                        ps[:, i, :],
                        kT8_s[:, :, kt * 128 : (kt + 1) * 128],
                        qT8_s[:, :, q0 : q0 + QB],
                        start=True,
                        stop=True,
                        perf_mode=DR,
                    )
                et = etp.tile([128, 2, QB], bf16, tag="et")
                nc.scalar.activation(out=et[:], in_=ps[:], func=Exp,
                                     scale=EXP_SCALE)
                pending_ec.append((blk, kp, et))

            def drain_ec(n=None):
                """emit n (default all) pending ec center-casts."""
                m = len(pending_ec) if n is None else min(n, len(pending_ec))
                for _ in range(m):
                    blk, kp, et = pending_ec.pop(0)
                    dst = ec8_s[:, blk, 2 * kp : 2 * kp + 2, :]
                    eng = ec_pat[blk][kp]
                    if eng == "A":
                        nc.scalar.activation(out=dst, in_=et[:], func=Copy,
                                             scale=SE, bias=-SE)
                    elif eng == "P":
                        nc.gpsimd.tensor_scalar(out=dst, in0=et[:], scalar1=SE,
                                                scalar2=-SE, op0=MULT, op1=ADD)
                    else:
                        nc.vector.tensor_scalar(out=dst, in0=et[:], scalar1=SE,
                                                scalar2=-SE, op0=MULT, op1=ADD)

            po_open = {}

            def out_half(blk, qs, part, split_dma=False):
                """Half of po's accumulation: part 0 = kp 0..7, part 1 =
                kp 8..15 + DVE epilogue (corr add, reciprocal, scale+bias,
                DMA out)."""
                if part == 0:
                    po = ps_o.tile([128, OC], f32, tag="ps_o")
                    po_open[(blk, qs)] = po
                    kps = range(0, NKP // 2)
                    first = True
                else:
                    po = po_open.pop((blk, qs))
                    kps = range(NKP // 2, NKP)
                    first = False
                for vs in (v8_s, vr8_s):
                    for kp in kps:
                        nc.tensor.matmul(
                            po[:],
                            ec8_s[:, blk, 2 * kp : 2 * kp + 2,
                                  qs * 128 : (qs + 1) * 128],
                            vs[:, 2 * kp : 2 * kp + 2, 0:OC],
                            start=first,
                            stop=(part == 1 and vs is vr8_s and kp == NKP - 1),
                            perf_mode=DR,
                        )
                        first = False
                if part == 0:
                    return
                t1 = t1p.tile([128, OC], f32, tag="t1")
                nc.vector.tensor_tensor(out=t1[:], in0=po[:], in1=corrb_s[:],
                                        op=ADD)
                rec = tiny.tile([128, 1], f32, tag="rec")
                nc.vector.reciprocal(rec[:], t1[:, 256:257])
                ot = outp.tile([128, D], f32, tag="ot")
                row = blk * (QB // 128) + qs
                if not split_dma:
                    nc.vector.scalar_tensor_tensor(
                        out=ot[:], in0=t1[:, 0:D], scalar=rec[:],
                        in1=aux_s[:, 1 : 1 + D], op0=MULT, op1=ADD)
                    nc.sync.dma_start(out[row], ot[:])
                else:
                    # final tiles: halve the exposed epilogue+DMA tail by
                    # overlapping the two column halves on two queues
                    nc.vector.scalar_tensor_tensor(
                        out=ot[:, 0:128], in0=t1[:, 0:128], scalar=rec[:],
                        in1=aux_s[:, 1:129], op0=MULT, op1=ADD)
                    nc.scalar.dma_start(out[row, :, 0:128], ot[:, 0:128])
                    nc.vector.scalar_tensor_tensor(
                        out=ot[:, 128:D], in0=t1[:, 128:D], scalar=rec[:],
                        in1=aux_s[:, 129 : 1 + D], op0=MULT, op1=ADD)
                    nc.sync.dma_start(out[row, :, 128:D], ot[:, 128:D])

            # ---- emission order (software pipeline) ----
            # Prime the Act Exp table at t=0 so the first real exp does not
            # pay the 1283ns table load on the critical path.
            warm = tiny.tile([128, 1], f32, tag="warm")
            nc.vector.memset(warm[:], 0.0)
            warm2 = tiny.tile([128, 1], bf16, tag="warm2")
            nc.scalar.activation(out=warm2[:], in_=warm[:], func=Exp)

            # critical-path DMAs first: wq split so the first matmul only
            # waits on the w8 half; weights on the SP HWDGE queue, x tiles
            # on the Act queue so the transfers overlap.
            nc.sync.dma_start(wq_s[:], wq[:])
            nc.sync.dma_start(bqk_s[:], bqk[:])
            nc.sync.dma_start(wk_s[:], wk[:])

            # Pair schedule: a pair (blk, kp) may be emitted once k-conv
            # tile kp//2 is done (and q-conv tile blk).  Pairs are spread so
            # Act's exp demand tracks each section's wall: 20 in A/B (blocks
            # 0 + early 1), 20 in C (blocks 1 + early 2), 24 in D.
            # iteration j of A/B: [leader (already-enabled pair, emitted
            # between q-conv and k-conv), new pairs (0,2j), (0,2j+1)]
            sched_ab = [
                [(0, 0), (0, 1)],
                [(1, 0), (0, 2), (0, 3)],
                [(1, 1), (0, 4), (0, 5)],
                [(1, 2), (0, 6), (0, 7)],
                [(1, 3), (0, 8), (0, 9)],
                [(1, 4), (0, 10), (0, 11)],
                [(1, 5), (0, 12), (0, 13)],
                [(1, 6), (0, 14), (0, 15)],
            ]
            sched_c = [
                [(1, 7), (1, 8)],
                [(1, 9), (1, 10), (1, 11)],
                [(1, 12), (1, 13)],
                [(1, 14), (1, 15), (2, 0)],
                [(2, 1), (2, 2)],
                [(2, 3), (2, 4), (2, 5)],
                [(2, 6), (2, 7)],
                [(2, 8), (2, 9), (2, 10)],
            ]
            sched_d = ([(2, kp) for kp in range(11, NKP)]
                       + [(3, kp) for kp in range(NKP)])

            cvT_cm = tc.tile_pool(name="ps_ab_cv", bufs=4, space="PSUM")
            ab_sc = tc.tile_pool(name="ps_ab_sc", bufs=2, space="PSUM")
            cvT = cvT_cm.__enter__()
            cur_sc[0] = ab_sc.__enter__()
            if True:
                conv_T(xq, wq_s, bqk_s[:, 0:2], qT8_s, 0, cvT,
                       dma_eng=nc.scalar, split_first=True)
                conv_T(xk, wk_s, bqk_s[:, 2:4], kT8_s, 0, cvT,
                       dma_eng=nc.scalar)
                for p in sched_ab[0]:
                    scores_pair(*p)
                drain_ec(1)
                for j in range(1, 4):
                    conv_T(xq, wq_s, bqk_s[:, 0:2], qT8_s, j, cvT,
                           dma_eng=nc.scalar)
                    scores_pair(*sched_ab[j][0])  # leader: old k tiles only
                    conv_T(xk, wk_s, bqk_s[:, 2:4], kT8_s, j, cvT)
                    for p in sched_ab[j][1:]:
                        scores_pair(*p)
                    drain_ec(len(sched_ab[j]))
                nc.sync.dma_start(wv_s[:], wv[:])
                nc.sync.dma_start(aux_s[:], aux[:])
                xv_fetch(0)
                xv_fetch(1)
                nc.sync.dma_start(ones8_s[:], ones8[:])
                nc.sync.dma_start(v8_s[:, :, D:VW], vtop8[:])
                nc.sync.dma_start(vr8_s[:, :, D:VW], vtop0[:])
                for j in range(4, 8):
                    scores_pair(*sched_ab[j][0])
                    conv_T(xk, wk_s, bqk_s[:, 2:4], kT8_s, j, cvT)
                    for p in sched_ab[j][1:]:
                        scores_pair(*p)
                    drain_ec(len(sched_ab[j]))
                xv_fetch(2)
                xv_fetch(3)
            ab_sc.__exit__(None, None, None)
            cvT_cm.__exit__(None, None, None)

            # C: v conv + pairs (blocks 1-2); their ec casts partially
            # deferred into D.  Once v8/vr8 tiles 0..15 exist (after conv_v
            # j3), block-0 part0 out-halves start filling PE's idle slots.
            ps_o_cm = tc.tile_pool(name="ps_o", bufs=CFG["ps_o_bufs"],
                                   space="PSUM")
            cvV_cm = tc.tile_pool(name="ps_cvV", bufs=CFG["ps_cv_bufs"],
                                  space="PSUM")
            scC_cm = tc.tile_pool(name="ps_c_sc", bufs=2, space="PSUM")
            ps_o = ps_o_cm.__enter__()
            cvV = cvV_cm.__enter__()
            scC = scC_cm.__enter__()
            if True:
                cur_sc[0] = scC
                for j in range(8):
                    sc = sched_c[j]
                    scores_pair(*sc[0])
                    conv_v(j, cvV)
                    for p in sc[1:]:
                        scores_pair(*p)
                    drain_ec(2)
                    if j == 5:
                        out_half(0, 0, 0)
                    if j == 7:
                        out_half(0, 1, 0)

                # correction: colsum(v8+vr8) replicated over rows, scaled by SE
                pc = cvV.tile([128, CT], f32, tag="ps_cv")
                first = True
                for vs in (v8_s, vr8_s):
                    for kp in range(NKP):
                        nc.tensor.matmul(pc[:, 0:OC], ones8_s[:],
                                         vs[:, 2 * kp : 2 * kp + 2, 0:OC],
                                         start=first,
                                         stop=(vs is vr8_s and kp == NKP - 1),
                                         perf_mode=DR)
                        first = False
                nc.vector.tensor_scalar(out=corrb_s[:], in0=pc[:, 0:OC],
                                        scalar1=SE, scalar2=None, op0=MULT)

            # D: out halves with the remaining pairs interleaved singly (a
            # lone pair between halves never outruns Act by more than the
            # ps_sc depth).  The staggered part0/part1 chain keeps exactly
            # two po tiles open (= ps_o bufs); block-3 part1 halves (which
            # need pair (3,15)) come last.
            scC_cm.__exit__(None, None, None)
            cvV_cm.__exit__(None, None, None)
            ps_d = tc.tile_pool(name="ps_d", bufs=3, space="PSUM")
            cur_sc[0] = ps_d.__enter__()
            di = 0

            def dpair():
                nonlocal di
                if di < len(sched_d):
                    scores_pair(*sched_d[di])
                    di += 1
                    drain_ec(2)

            chain = [
                ((0, 0, 1), (0, 2, 0)), ((0, 1, 1), (0, 3, 0)),
                ((0, 2, 1), (1, 0, 0)), ((0, 3, 1), (1, 1, 0)),
                ((1, 0, 1), (1, 2, 0)), ((1, 1, 1), (1, 3, 0)),
                ((1, 2, 1), (2, 0, 0)), ((1, 3, 1), (2, 1, 0)),
                ((2, 0, 1), (2, 2, 0)), ((2, 1, 1), (2, 3, 0)),
                ((2, 2, 1), (3, 0, 0)), ((2, 3, 1), (3, 1, 0)),
            ]
            for h1, h2 in chain:
                dpair()
                out_half(*h1)
                dpair()
                out_half(*h2)
            for _ in range(len(sched_d) - di):
                dpair()
            drain_ec()
            ps_d.__exit__(None, None, None)
            out_half(3, 0, 1)
            out_half(3, 2, 0)
            out_half(3, 1, 1)
            out_half(3, 3, 0)
            out_half(3, 2, 1)
            out_half(3, 3, 1)
            ps_o_cm.__exit__(None, None, None)

    _split_drain_waits(nc)
    return nc


_NC_CACHE = None


def _get_nc():
    global _NC_CACHE
    if _NC_CACHE is None:
        _NC_CACHE = _build_bass()
    return _NC_CACHE


def _fp8(a):
    import ml_dtypes
    return np.asarray(np.clip(a, -240.0, 240.0), ml_dtypes.float8_e4m3)


def _xtiles(x_pad):
    """[128, 2, n+2] f32 -> fp8 2-term tiles [nj, 128, 4, 528]."""
    n = x_pad.shape[2] - 2
    nj = n // CT
    x8 = _fp8(x_pad * SX)
    xr8 = _fp8(x_pad * SX - x8.astype(np.float32))
    tiles = np.zeros((nj, 128, 4, XT_ST), x8.dtype)
    for j in range(nj):
        sl = slice(j * CT, j * CT + CT + 2)
        tiles[j, :, 0:2, 0 : CT + 2] = x8[:, :, sl]
        tiles[j, :, 2:4, 0 : CT + 2] = xr8[:, :, sl]
    return tiles


def _xT_padded(x_b):
    """[S, C] -> transposed + halo-padded [128, 2, S+2] f32."""
    xt = np.zeros((DIN, x_b.shape[0] + 2), np.float32)
    xt[:, 1:-1] = x_b.T
    return np.ascontiguousarray(
        xt.reshape(2, 128, x_b.shape[0] + 2).transpose(1, 0, 2)
    )


def _w2(w_scaled):
    """scaled f32 weights -> (w8, wr8) fp8 pair."""
    w8 = _fp8(w_scaled)
    wr8 = _fp8(w_scaled - w8.astype(np.float32))
    return w8, wr8


def _prep_shared(q_w, q_b, k_w, k_b, v_w, v_b, scale):
    import ml_dtypes
    FP8 = ml_dtypes.float8_e4m3

    def w_T(w):  # [co, ci, 3] -> [p, a, t, o, m] f32
        arr = np.ascontiguousarray(w.transpose(1, 2, 0))  # [ci, t, co]
        arr = arr.reshape(2, 128, 3, 2, 128)  # [a, p, t, o, m]
        return np.ascontiguousarray(arr.transpose(1, 0, 2, 3, 4)).astype(np.float32)

    def w_v(w):  # [co, ci, 3] -> [p, a, t, co] f32
        arr = np.ascontiguousarray(w.transpose(1, 2, 0))
        arr = arr.reshape(2, 128, 3, D)
        return np.ascontiguousarray(arr.transpose(1, 0, 2, 3)).astype(np.float32)

    def pack_T(w):
        return _fp8(w_T(w) * SW)

    wv8, wvr8 = _w2(w_v(v_w) * SW)
    vtop8 = np.zeros((128, NKT, VW - D), FP8)
    vtop8[:, :, 0] = FP8(SV)
    return {
        "wq": pack_T(q_w),
        "wk": pack_T(k_w),
        "wv": np.ascontiguousarray(np.stack([wv8, wvr8], axis=1)),
        "bqk": np.concatenate([
            np.ascontiguousarray(q_b.reshape(2, 128).T).astype(np.float32),
            np.ascontiguousarray(k_b.reshape(2, 128).T).astype(np.float32),
        ], axis=1) * SQK,
        "aux": np.concatenate([
            np.full((128, 1), float(scale) * 2.0**-11, np.float32),
            np.tile(v_b.astype(np.float32)[None, :] * float(scale), (128, 1)),
        ], axis=1),
        "ones8": np.ones((128, 2, 128), FP8),
        "vtop8": vtop8,
        "vtop0": np.zeros((128, NKT, VW - D), FP8),
    }


def kernel(query, key, value, q_w, q_b, k_w, k_b, v_w, v_b, scale):
    from concourse.bass_utils import run_bass_kernel_spmd

    query = np.asarray(query, np.float32)
    key = np.asarray(key, np.float32)
    value = np.asarray(value, np.float32)

    shared = _prep_shared(
        np.asarray(q_w), np.asarray(q_b), np.asarray(k_w), np.asarray(k_b),
        np.asarray(v_w), np.asarray(v_b), np.asarray(scale),
    )

    in_maps = []
    for c in range(NCORES):
        b, h = c // 2, c % 2
        xq_full = _xT_padded(query[b])  # [128, 2, S+2]
        xq_c = np.ascontiguousarray(xq_full[:, :, h * SQ : h * SQ + SQ + 2])
        m = dict(shared)
        m["xq"] = _xtiles(xq_c)
        m["xk"] = _xtiles(_xT_padded(key[b]))
        m["xv"] = _xtiles(_xT_padded(value[b]))
        in_maps.append(m)

    nc = _get_nc()
    res = run_bass_kernel_spmd(nc, in_maps, core_ids=list(range(NCORES)))

    out_full = np.empty((B, S, D), np.float32)
    for c in range(NCORES):
        b, h = c // 2, c % 2
        out_full[b, h * SQ : (h + 1) * SQ, :] = res.results[c]["out"].reshape(SQ, D)
    return out_full
